# revision 67
# baseline (speedup 1.0000x reference)
"""Trainium2 Bass kernel for nn_DetectorKe_652835029279 (Gaussian-mixture
log-likelihood detector: weighted logsumexp over 256 Mahalanobis distances).

Math: ll_i = logsumexp_j( -0.5 x^T A_j x + x . (A_j c_j) + bias_j )
with bias_j = log(coef_j) - 0.5 c_j^T A_j c_j - thr folded in. The quadratic
expands over cyclic-rotation pair blocks (d, (d+k) % 32):
  chunks 0..3 (bf16): k = 1..16 pair products (512 slots, off-diag, doubled
    coefficients except k=16) = xt4 * rot, one broadcast DVE multiply;
  chunk 4 (bf16): [diag squares x_d^2 (32); x_d linear (32); 1 bias row;
    zero pad] - squares via DVE bf16, linear via GpSimd copy, statics once.
X is fed as bf16 (half the HBM traffic, 1 cyc/row transposes, all-bf16
PSUM->SBUF cast on DVE). xt4 = K=128 sel matmul + scalar copy. Rotations via
4 concurrent K=32 row-tiled sel matmuls (tile_position=(32g,0)).
Main matmul: per 128-row subtile, 5 accumulating matmuls, stationary = bf16
G-chunk [128,128] (FWL), moving = bf16 U [128,256]; exp + free-dim
accumulate on ACT per subtile.
Software-pipelined 2 tiles ahead: construction of tile t+2 is emitted before
tile t's main matmuls so the cast->sel->mult chain has two periods to run.
"""
import os
import sys

if "/opt/trn_rl_repo" not in sys.path:
    sys.path.insert(0, "/opt/trn_rl_repo")
# A prior wedged run can leave the NeuronCores in a ~17%-degraded state;
# a core reset on open restores full clocks and costs nothing measurable.
os.environ.setdefault("NEURON_RT_RESET_CORES", "1")

import numpy as np

N, D, M = 131072, 32, 256
NCORES = 8
NC_ROWS = N // NCORES          # 16384
TILE_ROWS = 512
NTILES = NC_ROWS // TILE_ROWS  # 32
NGROUPS = NC_ROWS // 128       # 128
NCHUNK = 5
KL = [[1, 2, 3, 4], [5, 6, 7, 8], [9, 10, 11, 12], [13, 14, 15, 16]]

_PROGRAM = None


def _build_program():
    import concourse.bacc as bacc
    import concourse.mybir as mybir
    import concourse.tile as tile

    f32 = mybir.dt.float32
    bf16 = mybir.dt.bfloat16
    fp8 = mybir.dt.float8e4
    AF = mybir.ActivationFunctionType
    DR = mybir.MatmulPerfMode.DoubleRow

    nc = bacc.Bacc(None, target_bir_lowering=False)
    X_d = nc.dram_tensor("X", [NC_ROWS, D], bf16, kind="ExternalInput")
    U8_d = nc.dram_tensor("U8", [128, 2, 2, M], fp8, kind="ExternalInput")
    U4_d = nc.dram_tensor("U4", [128, M], bf16, kind="ExternalInput")
    SEL0_d = nc.dram_tensor("SEL0", [128, 128], bf16, kind="ExternalInput")
    SELR_d = nc.dram_tensor("SELR", [128, 128], bf16, kind="ExternalInput")
    EYEB_d = nc.dram_tensor("EYEB", [128, 128], bf16, kind="ExternalInput")
    OUT_d = nc.dram_tensor("out", [128, NGROUPS], f32, kind="ExternalOutput")

    with tile.TileContext(nc) as tc:
        with (
            tc.tile_pool(name="const", bufs=1) as constp,
            tc.tile_pool(name="xin", bufs=4) as xinp,
            tc.tile_pool(name="xtp", bufs=3) as xtpool,
            tc.tile_pool(name="xt4p", bufs=3) as xt4pool,
            tc.tile_pool(name="ch4p", bufs=3) as ch4pool,
            tc.tile_pool(name="xxp", bufs=3) as xxpool,
            tc.tile_pool(name="expp", bufs=4) as exppool,
            tc.tile_pool(name="sumsp", bufs=1) as sumspool,
            tc.tile_pool(name="finp", bufs=1) as finpool,
            tc.tile_pool(name="ps_xt", bufs=1, space="PSUM") as ps_xt,
            tc.tile_pool(name="ps_xt4", bufs=1, space="PSUM") as ps_xt4,
            tc.tile_pool(name="ps_rot", bufs=2, space="PSUM") as ps_rot,
            tc.tile_pool(name="ps_main", bufs=2, space="PSUM") as ps_main,
        ):
            U8_sb = constp.tile([128, 2, 2, M], fp8)
            U4_sb = constp.tile([128, M], bf16)
            SEL0_sb = constp.tile([128, 128], bf16)
            SELR_sb = constp.tile([128, 128], bf16)
            EYEB_sb = constp.tile([128, 128], bf16)
            # EYEB first: the warm-up matmuls and first transposes need it
            nc.sync.dma_start(EYEB_sb[:], EYEB_d[:])

            sums_sb = sumspool.tile([128, NGROUPS], f32)

            # warm-up: ~4us of junk matmuls raises the PE HAM clock to 2.4GHz
            # and preloads the exp ACT table while the first X tiles DMA in.
            warm_ps = ps_xt4.tile([128, 128], f32, tag="xt4ps")
            for i in range(18):
                nc.tensor.matmul(
                    warm_ps[:], EYEB_sb[:], EYEB_sb[:], start=True, stop=True
                )
            warm_sb = finpool.tile([128, 1], f32)
            nc.scalar.activation(warm_sb[:], warm_ps[:, 0:1], AF.Exp)

            # persistent tiles: static regions written once; per-tile writes
            # only touch the dynamic rows so statics stay valid.
            xt_tiles = []
            for i in range(3):
                xt_p = xtpool.tile(
                    [128, TILE_ROWS], bf16, tag=f"xtP{i}", bufs=1, name=f"xt_p{i}"
                )
                for lo in (32, 64, 96):  # sel pad rows (quadrant-aligned)
                    nc.gpsimd.memset(xt_p[lo : lo + 32, :], 0.0)
                xt_tiles.append(xt_p)
            ch4_tiles = []
            for i in range(3):
                ch4_p = ch4pool.tile(
                    [128, TILE_ROWS], bf16, tag=f"ch4P{i}", bufs=1, name=f"ch4_p{i}"
                )
                nc.gpsimd.memset(ch4_p[64:96, :], 0.0)   # zero pad (U rows are 0)
                nc.gpsimd.memset(ch4_p[96:128, :], 0.0)
                nc.gpsimd.memset(ch4_p[64:65, :], 1.0)   # bias row
                ch4_tiles.append(ch4_p)

            def stage0(t):
                """Issue the X-tile DMA (longest latency, 3 tiles ahead)."""
                x_t = xinp.tile([128, 4 * D], bf16, tag="x")
                nc.sync.dma_start(
                    x_t[:].rearrange("p (g d) -> p g d", g=4),
                    X_d[t * TILE_ROWS : (t + 1) * TILE_ROWS, :].rearrange(
                        "(g p) d -> p g d", p=128
                    ),
                )
                return x_t

            def stageA(t, x_t):
                """Transpose + cast (2 tiles ahead)."""
                xtps = ps_xt.tile([32, TILE_ROWS], bf16, tag="xtps")
                for g in range(4):
                    nc.tensor.transpose(
                        xtps[:, g * 128 : (g + 1) * 128],
                        x_t[:, g * D : (g + 1) * D],
                        EYEB_sb[:],
                    )
                xt_sb = xt_tiles[t % 3]
                nc.vector.tensor_copy(xt_sb[0:32, :], xtps[:])  # bf16 2x cast
                return xt_sb

            def stageB(t, xt_sb):
                """Sel matmuls + products (1 tile ahead of main)."""
                # xt4 = 4-fold stack of X^T (partition p holds x_{p%32})
                xt4ps = ps_xt4.tile([128, TILE_ROWS], f32, tag="xt4ps")
                nc.tensor.matmul(
                    xt4ps[:], SEL0_sb[:], xt_sb[:], start=True, stop=True
                )
                xt4_sb = xt4pool.tile([128, TILE_ROWS], bf16, tag="xt4")
                nc.scalar.copy(xt4_sb[:], xt4ps[:])

                # chunk 4 dynamic rows: squares (gpsimd) + linear (DMA copy)
                ch4 = ch4_tiles[t % 3]
                nc.gpsimd.tensor_mul(ch4[0:32, :], xt_sb[0:32, :], xt_sb[0:32, :])
                nc.sync.dma_start(ch4[32:64, :], xt_sb[0:32, :])

                # rotation sels (K=32 row-tiled, concurrent pairs) + pair
                # products, in 2 double-buffered half-passes so next tile's
                # sels overlap this tile's multiply:
                # rot_g[p] = x_{(p%32 + KL[g][p//32]) % 32}
                xx = xxpool.tile([128, 4, TILE_ROWS], fp8, tag="xx")
                for h in range(2):
                    rotq = ps_rot.tile([128, 2, TILE_ROWS], f32, tag="rotq")
                    for g2 in range(2):
                        g = 2 * h + g2
                        nc.tensor.matmul(
                            rotq[:, g2, :],
                            SELR_sb[32 * g : 32 * (g + 1), :],
                            xt4_sb[32 * g : 32 * (g + 1), :],
                            start=True,
                            stop=True,
                            tile_position=(32 * g, 0),
                        )
                    nc.vector.tensor_mul(
                        xx[:, 2 * h : 2 * h + 2, :],
                        rotq[:],
                        xt4_sb[:, None, :].broadcast_to([128, 2, TILE_ROWS]),
                    )
                return xx, ch4

            def main_phase(t, xx, ch4):
                for half in range(2):
                    psmain = ps_main.tile([128, 2 * M], f32, tag="main")
                    for s2 in range(2):
                        sub = half * 2 + s2
                        # 2 DoubleRow fp8 matmuls (chunks 0+1, 2+3: K=256
                        # each, 2 weights/cell) + 1 bf16 matmul (chunk 4),
                        # then exp immediately so ACT overlaps the next group
                        for P in range(2):
                            nc.tensor.matmul(
                                psmain[:, s2 * M : (s2 + 1) * M],
                                xx[:, 2 * P : 2 * P + 2, sub * 128 : (sub + 1) * 128],
                                U8_sb[:, P, :, :],
                                start=(P == 0),
                                stop=False,
                                perf_mode=DR,
                            )
                        nc.tensor.matmul(
                            psmain[:, s2 * M : (s2 + 1) * M],
                            ch4[:, sub * 128 : (sub + 1) * 128],
                            U4_sb[:],
                            start=False,
                            stop=True,
                        )
                    for s2 in range(2):
                        sub = half * 2 + s2
                        expsc = exppool.tile([128, M], f32, tag="exp")
                        col = t * 4 + sub
                        nc.scalar.activation(
                            expsc[:],
                            psmain[:, s2 * M : (s2 + 1) * M],
                            AF.Exp,
                            accum_out=sums_sb[:, col : col + 1],
                        )

            # software pipeline: DMA 3 ahead, transpose+cast 2 ahead,
            # sels+products 1 ahead of the main matmul phase. Remaining
            # const DMAs are interleaved on the sync queue in need-order
            # (X0 before the sel matrices, U before the first main phase).
            xq = [stage0(0)]
            nc.sync.dma_start(SEL0_sb[:], SEL0_d[:])
            nc.sync.dma_start(SELR_sb[:], SELR_d[:])
            xq += [stage0(1), stage0(2)]
            nc.sync.dma_start(U8_sb[:], U8_d[:])
            nc.sync.dma_start(U4_sb[:], U4_d[:])
            aq = [stageA(0, xq[0]), stageA(1, xq[1])]
            bq = [stageB(0, aq[0])]
            for t in range(NTILES):
                if t + 3 < NTILES:
                    xq.append(stage0(t + 3))
                if t + 2 < NTILES:
                    aq.append(stageA(t + 2, xq[t + 2]))
                if t + 1 < NTILES:
                    bq.append(stageB(t + 1, aq[t + 1]))
                main_phase(t, *bq[t])

            # epilogue: DMA out the raw exp-sums; the host takes the log
            # (saves the Ln ACT-table swap + transpose + copy in the tail)
            nc.sync.dma_start(OUT_d[:], sums_sb[:])

    nc.compile()
    return nc


def _host_prep(center, cov_inv_sqrt, weight, threshold):
    import ml_dtypes

    bf = ml_dtypes.bfloat16
    L = np.asarray(cov_inv_sqrt, dtype=np.float64)
    w = np.abs(np.asarray(weight, dtype=np.float64))
    pr = w / w.sum()
    A = np.einsum("mij,mkj->mik", L, L)
    sign, logdet = np.linalg.slogdet(A)
    logcoef = np.log(pr) + 0.5 * logdet
    c64 = np.asarray(center, dtype=np.float64)
    Ac = np.einsum("mkl,ml->mk", A, c64)
    term3 = np.einsum("mk,mk->m", c64, Ac)
    bias = logcoef - 0.5 * term3 - float(np.asarray(threshold).reshape(-1)[0])

    p = np.arange(128)
    U = np.zeros((128, NCHUNK, M), np.float32)
    for c in range(4):
        k = np.array(KL[c])[p // 32]
        a = p % 32
        b = (a + k) % 32
        mult = np.where(k == 16, 1.0, 2.0)
        U[:, c, :] = -0.5 * mult[:, None] * A[:, a, b].T
    d32 = np.arange(32)
    U[0:32, 4, :] = -0.5 * A[:, d32, d32].T
    U[32:64, 4, :] = Ac.T
    U[64, 4, :] = bias
    U8 = np.ascontiguousarray(
        U[:, :4, :].reshape(128, 2, 2, M).astype(ml_dtypes.float8_e4m3)
    )
    U4 = np.ascontiguousarray(U[:, 4, :].astype(bf))

    dd = np.arange(128)
    SEL0 = (dd[:, None] == (p[None, :] % 32)).astype(bf)
    SELR = np.zeros((128, 128), np.float32)
    for g in range(4):
        k = np.array(KL[g])[p // 32]
        b = (p % 32 + k) % 32
        SELR[32 * g : 32 * (g + 1), :] = (
            np.arange(32)[:, None] == b[None, :]
        ).astype(np.float32)
    SELR = SELR.astype(bf)
    EYEB = np.eye(128, dtype=np.float32).astype(bf)
    return U8, U4, SEL0, SELR, EYEB


def kernel(X, center, cov_inv_sqrt, weight, threshold):
    global _PROGRAM
    import ml_dtypes
    from concourse.bass_utils import run_bass_kernel_spmd

    Xb = np.ascontiguousarray(
        np.asarray(X, dtype=np.float32).astype(ml_dtypes.bfloat16)
    )
    U8, U4, SEL0, SELR, EYEB = _host_prep(center, cov_inv_sqrt, weight, threshold)

    if _PROGRAM is None:
        _PROGRAM = _build_program()
    nc = _PROGRAM

    in_maps = []
    for k in range(NCORES):
        in_maps.append(
            {
                "X": Xb[k * NC_ROWS : (k + 1) * NC_ROWS],
                "U8": U8,
                "U4": U4,
                "SEL0": SEL0,
                "SELR": SELR,
                "EYEB": EYEB,
            }
        )
    res = run_bass_kernel_spmd(nc, in_maps, list(range(NCORES)))
    # device returns exp-sums [128 partitions, 128 row-groups]; row index is
    # group*128 + partition, and ll = log(sum) (threshold folded into bias)
    out = np.concatenate(
        [
            np.log(res.results[k]["out"].astype(np.float64)).T.ravel()
            for k in range(NCORES)
        ]
    )
    return out.astype(np.float32)
